# revision 4
# baseline (speedup 1.0000x reference)
"""Trainium2 Bass kernel for DeepDeltaResidualExpanded.

out = x + k_rms[..., :, None] * delta[..., None, :]
  k_rms = rmsnorm(k_in);  beta = 2*sigmoid(ctx @ bw.T + bb)
  proj = einsum('btd,btdv->btv', k_rms, x) * k_scale
  v    = sigmoid(v_in @ vw.T + vb) * 4
  delta = beta * (v - proj) * k_scale

Pure data parallel over B*T rows across 8 NeuronCores; the tiny
beta/v weights are replicated.

Engine budget per 128-row subtile (DMA window is ~16 us at the
358 GB/s per-core HBM limit):
  - DVE: gate products as bf16 tensor_tensor (2x_1p mode), pv and the
    final update as fp32 stt (1x, no accel exists for 2-tensor fp32),
    plus a few small ops  (~13.5 us)
  - ACT: all transcendentals drawn from ONE table set
    (natural_log_exp_and_others: square/ln/exp/identity/copy) so no
    per-tile ACT_TABLE_LOAD churn; the gate reductions run here as
    Identity-with-accum over the DVE products (~7 us)
  - sigmoid(z) is computed as 1/(1+exp(-z)) (exp on ACT, reciprocal on
    DVE) and 1/sqrt(s) as exp(-0.5*ln(s)) to stay inside that one set.
Loads: x,k fp32 on the SP HWDGE ring; c,v as SWDGE cast-to-bf16 loads
(HBM still reads fp32; SBUF gets bf16 for the 2x gate products);
stores on the ACT HWDGE ring.  Two 128-row subtiles per DMA transfer
so every transfer is >= 1 MiB.
"""

import numpy as np

B, T, D, DV = 4, 4096, 1024, 4
N_CORES = 8
ROWS = B * T
ROWS_PER_CORE = ROWS // N_CORES  # 2048
P = 128
S = 2  # subtiles per supertile (DMA granularity = S*128 rows)

K_EPS = 1e-05
V_SIG_SCALE = 4.0
# C = k_scale / sqrt(mean(k^2) + eps_rms) == 1/sqrt(sum_d k^2 + 1e-10)
SQRT_BIAS = K_EPS * K_EPS  # 1e-10
LN2 = 0.6931471805599453


def _build_nc(rows, repeat=1):
    """Build + compile the single-core Bass program for `rows` rows.

    repeat > 1 wraps the whole body in a HW loop that redoes identical
    work — only used by the benchmark harness to lift device time above
    host dispatch noise; results are unchanged (idempotent body).
    """
    import contextlib

    import concourse.bacc as bacc
    import concourse.mybir as mybir
    import concourse.tile as tile
    from concourse.bass import AP

    f32 = mybir.dt.float32
    bf16 = mybir.dt.bfloat16
    Alu = mybir.AluOpType
    Act = mybir.ActivationFunctionType
    assert rows % (P * S) == 0
    nsuper = rows // (P * S)

    nc = bacc.Bacc("TRN2", target_bir_lowering=False, debug=False)

    x_d = nc.dram_tensor("x", [rows, D * DV], f32, kind="ExternalInput")
    k_d = nc.dram_tensor("k", [rows, D], f32, kind="ExternalInput")
    v_d = nc.dram_tensor("v", [rows, D], f32, kind="ExternalInput")
    c_d = nc.dram_tensor("c", [rows, D], f32, kind="ExternalInput")
    bw_d = nc.dram_tensor("bw", [1, D], f32, kind="ExternalInput")
    bb_d = nc.dram_tensor("bb", [1, 1], f32, kind="ExternalInput")
    vw_d = nc.dram_tensor("vw", [DV, D], f32, kind="ExternalInput")
    vb_d = nc.dram_tensor("vb", [1, DV], f32, kind="ExternalInput")
    y_d = nc.dram_tensor("y", [rows, D * DV], f32, kind="ExternalOutput")

    def pbcast(handle, shape=None):
        # Read the same DRAM bytes into all 128 partitions (step-0 AP).
        ap = handle.ap()
        return AP(tensor=ap.tensor, offset=ap.offset, ap=[[0, P], *ap.ap])

    def dram_rows(handle, r0, width):
        # [P, S, width] view of S*128 DRAM rows: partition p, seg s ->
        # row r0 + s*128 + p.
        return (
            handle.ap()[r0 : r0 + P * S, :].rearrange("(s p) f -> p s f", s=S)
        )

    with tile.TileContext(nc) as tc:
        with (
            tc.tile_pool(name="consts", bufs=1) as consts,
            tc.tile_pool(name="xp", bufs=3) as xp,
            tc.tile_pool(name="kp", bufs=3) as kp,
            tc.tile_pool(name="cvp", bufs=3) as cvp,
            tc.tile_pool(name="scrb", bufs=3) as scrb,
            tc.tile_pool(name="scrf", bufs=2) as scrf,
            tc.tile_pool(name="smallp", bufs=4) as smallp,
        ):
            # bf16 copies of the gate weights, broadcast to all partitions
            # (SWDGE dma casts fp32->bf16 in flight).
            bw_b = consts.tile([P, D], bf16)
            nc.gpsimd.dma_start(out=bw_b[:], in_=pbcast(bw_d))
            vw_b = consts.tile([P, DV, D], bf16)
            nc.gpsimd.dma_start(out=vw_b[:], in_=pbcast(vw_d))
            bb_b = consts.tile([P, 1], f32)
            nc.gpsimd.dma_start(out=bb_b[:], in_=pbcast(bb_d))
            vb_b = consts.tile([P, DV], f32)
            nc.gpsimd.dma_start(out=vb_b[:], in_=pbcast(vb_d))
            # biases pre-scaled by 1/D: the gate reduction applies its bias
            # per element (accum_out = sum(in + bias) = sum(in) + D*bias).
            bbs = consts.tile([P, 1], f32)
            nc.scalar.activation(bbs[:], bb_b[:], Act.Copy, scale=1.0 / D)
            vbs = consts.tile([P, DV], f32)
            nc.scalar.activation(vbs[:], vb_b[:], Act.Copy, scale=1.0 / D)
            eps_t = consts.tile([P, 1], f32)
            nc.vector.memset(eps_t[:], SQRT_BIAS)
            ln2_t = consts.tile([P, 1], f32)
            nc.vector.memset(ln2_t[:], LN2)

            loop_cm = (
                tc.For_i(0, repeat, 1) if repeat > 1 else contextlib.nullcontext()
            )
            with loop_cm:
                for i in range(nsuper):
                    r0 = i * P * S
                    x_t = xp.tile([P, S, D * DV], f32)
                    nc.sync.dma_start(out=x_t[:], in_=dram_rows(x_d, r0, D * DV))
                    k_t = kp.tile([P, S, D], f32)
                    nc.sync.dma_start(out=k_t[:], in_=dram_rows(k_d, r0, D))
                    c_bf = cvp.tile([P, S, D], bf16, tag="c")
                    nc.gpsimd.dma_start(out=c_bf[:], in_=dram_rows(c_d, r0, D))
                    v_bf = cvp.tile([P, S, D], bf16, tag="v")
                    nc.gpsimd.dma_start(out=v_bf[:], in_=dram_rows(v_d, r0, D))

                    x4 = x_t.rearrange("p s (d v) -> p s d v", v=DV)

                    for t in range(S):
                        kk = k_t[:, t, :]
                        x3 = x4[:, t]

                        # --- C = 1/sqrt(sum k^2 + 1e-10) = exp(-0.5*ln(.))
                        sq = scrf.tile([P, D], f32, tag="sq")
                        ms = smallp.tile([P, 1], f32, tag="ms")
                        nc.scalar.activation(sq[:], kk, Act.Square, accum_out=ms[:])
                        lns = smallp.tile([P, 1], f32, tag="lns")
                        nc.scalar.activation(lns[:], ms[:], Act.Ln, bias=eps_t[:])
                        cc = smallp.tile([P, 1], f32, tag="cc")
                        nc.scalar.activation(cc[:], lns[:], Act.Exp, scale=-0.5)
                        cc2 = smallp.tile([P, 1], f32, tag="cc2")
                        nc.scalar.activation(
                            cc2[:], lns[:], Act.Exp, scale=-0.5, bias=ln2_t[:]
                        )

                        # --- gate logits: bf16 products on DVE (2x mode),
                        # reduction + bias on ACT (Identity with accum).
                        l5 = smallp.tile([P, 5], f32, tag="l5")
                        pb = scrb.tile([P, D], bf16, tag="pb")
                        nc.vector.tensor_mul(pb[:], c_bf[:, t, :], bw_b[:])
                        nc.scalar.activation(
                            pb[:], pb[:], Act.Identity, bias=bbs[:],
                            accum_out=l5[:, 0:1],
                        )
                        for j in range(DV):
                            pbj = scrb.tile([P, D], bf16, tag="pb")
                            nc.vector.tensor_mul(
                                pbj[:], v_bf[:, t, :], vw_b[:, j, :]
                            )
                            nc.scalar.activation(
                                pbj[:], pbj[:], Act.Identity,
                                bias=vbs[:, j : j + 1],
                                accum_out=l5[:, 1 + j : 2 + j],
                            )

                        # --- pv[j] = C * sum_d k*x_j (C folded as stt scalar)
                        pv4 = smallp.tile([P, DV], f32, tag="pv4")
                        for j in range(DV):
                            scr = scrf.tile([P, D], f32, tag="scr")
                            nc.vector.scalar_tensor_tensor(
                                out=scr[:], in0=kk, scalar=cc[:], in1=x3[:, :, j],
                                op0=Alu.mult, op1=Alu.mult,
                                accum_out=pv4[:, j : j + 1],
                            )

                        # --- sigmoids: sig = 1/(1 + exp(-logit))
                        e5 = smallp.tile([P, 5], f32, tag="e5")
                        nc.scalar.activation(e5[:], l5[:], Act.Exp, scale=-1.0)
                        t5 = smallp.tile([P, 5], f32, tag="t5")
                        nc.scalar.activation(t5[:], e5[:], Act.Copy, bias=1.0)
                        sg5 = smallp.tile([P, 5], f32, tag="sg5")
                        nc.vector.reciprocal(sg5[:], t5[:])

                        # --- gamma[j] = (2*sig_b*C) * (4*sig_v[j] - pv[j])
                        bc = smallp.tile([P, 1], f32, tag="bc")
                        nc.scalar.activation(
                            bc[:], sg5[:, 0:1], Act.Copy, scale=cc2[:]
                        )
                        w = smallp.tile([P, DV], f32, tag="w")
                        nc.vector.scalar_tensor_tensor(
                            out=w[:], in0=sg5[:, 1:5], scalar=V_SIG_SCALE,
                            in1=pv4[:], op0=Alu.mult, op1=Alu.subtract,
                        )
                        gm = smallp.tile([P, DV], f32, tag="gm")
                        nc.vector.tensor_scalar_mul(gm[:], w[:], bc[:])

                        # --- out_v = k*gamma_v + x_v (in place)
                        for j in range(DV):
                            nc.vector.scalar_tensor_tensor(
                                out=x3[:, :, j], in0=kk,
                                scalar=gm[:, j : j + 1], in1=x3[:, :, j],
                                op0=Alu.mult, op1=Alu.add,
                            )
                    # store via the second HWDGE ring (Activation) so queued
                    # stores never head-of-line block the load stream on SP
                    nc.scalar.dma_start(
                        out=dram_rows(y_d, r0, D * DV), in_=x_t[:]
                    )

    nc.compile()
    return nc


_NC_CACHE = {}


def _get_nc(rows):
    if rows not in _NC_CACHE:
        _NC_CACHE[rows] = _build_nc(rows)
    return _NC_CACHE[rows]


def _shard_inputs(inputs):
    x = np.ascontiguousarray(inputs["x"], dtype=np.float32).reshape(ROWS, D * DV)
    k = np.ascontiguousarray(inputs["k_in"], dtype=np.float32).reshape(ROWS, D)
    v = np.ascontiguousarray(inputs["v_in"], dtype=np.float32).reshape(ROWS, D)
    c = np.ascontiguousarray(inputs["context"], dtype=np.float32).reshape(ROWS, D)
    bw = np.ascontiguousarray(inputs["beta_w"], dtype=np.float32).reshape(1, D)
    bb = np.ascontiguousarray(inputs["beta_b"], dtype=np.float32).reshape(1, 1)
    vw = np.ascontiguousarray(inputs["v_w"], dtype=np.float32).reshape(DV, D)
    vb = np.ascontiguousarray(inputs["v_b"], dtype=np.float32).reshape(1, DV)
    in_maps = []
    for core in range(N_CORES):
        sl = slice(core * ROWS_PER_CORE, (core + 1) * ROWS_PER_CORE)
        in_maps.append(
            {"x": x[sl], "k": k[sl], "v": v[sl], "c": c[sl],
             "bw": bw, "bb": bb, "vw": vw, "vb": vb}
        )
    return in_maps


def kernel_run(inputs, trace=False):
    """Returns (full output array, BassKernelResults)."""
    from concourse.bass_utils import run_bass_kernel_spmd

    nc = _get_nc(ROWS_PER_CORE)
    in_maps = _shard_inputs(inputs)
    res = run_bass_kernel_spmd(
        nc, in_maps, core_ids=list(range(N_CORES)), trace=trace
    )
    y = np.concatenate([res.results[c]["y"] for c in range(N_CORES)], axis=0)
    return y.reshape(B, T, D, DV), res


def kernel(**inputs):
    out, _ = kernel_run(inputs)
    return out
